# revision 11
# baseline (speedup 1.0000x reference)
"""GATv2 layer kernel for Trainium2, sharded across 8 NeuronCores.

Computation (reference):
    Wh = h @ W.T                       [N, F]
    s1 = Wh @ a1, s2 = Wh @ a2         [N]
    e  = leaky_relu(s1[:,None] + s2[None,:], 0.2)
    attention = softmax(e * adj, dim=1)
    out = attention @ Wh               [N, F]

Sharding: rows (destination nodes) split across 8 cores, 1024 rows each.
Each core gets its adj row-block plus replicated h/W/a, computes its
1024x128 output block; host concatenates.

Per-core pipeline, tiled [128 rows x 2048 cols]:
    ACT : L = Prelu(SJbc + s1_row, alpha=0.2)      (per-partition bias)
    DVE : T = L * adj_tile
    PE  : transpose T 128x128 tiles -> PSUM [128,1024]
    ACT : P^T = Exp(T^T)  PSUM -> SBUF bf16        (fused evacuation)
    PE  : acc += P^T.T @ [Wh | 1] (bf16, FWL)      (ones col = softmax denom)
    DVE : out_rows = acc[:, :128] * 1/acc[:, 128]
Softmax is computed without max subtraction: scores are O(6) so exp is
safely in fp32 range, matching the reference up to fp rounding.
"""
import sys

for _p in ("/opt/trn_rl_repo", "/root/.axon_site/_ro/trn_rl_repo"):
    if _p not in sys.path:
        sys.path.insert(0, _p)

import numpy as np
from contextlib import ExitStack

from concourse import bacc, tile, mybir
from concourse.bass_utils import run_bass_kernel_spmd
from concourse.masks import make_identity

f32 = mybir.dt.float32
bf16 = mybir.dt.bfloat16
AL = mybir.AluOpType
AF = mybir.ActivationFunctionType

N = 8192
F = 128
NCORES = 8
RPC = N // NCORES          # rows per core = 1024
RT = RPC // 128            # row tiles per core = 8
CCH = 2048                 # column chunk for PRELU/MULT
NCH = N // CCH             # col chunks per row tile = 4
PSW = 1024                 # psum transpose tile width (2 banks)
NEG_SLOPE = 0.2

_CACHE = {}


def _build():
    nc = bacc.Bacc("TRN2", target_bir_lowering=False)

    adj_ext = nc.declare_dram_parameter("adj", [RPC, N], f32, isOutput=False)
    hT_ext = nc.declare_dram_parameter("hT", [F, N], f32, isOutput=False)
    hTloc_ext = nc.declare_dram_parameter("hT_loc", [F, RPC], f32, isOutput=False)
    wt_ext = nc.declare_dram_parameter("wt", [F, F], f32, isOutput=False)  # W^T [fi, fo]
    w_ext = nc.declare_dram_parameter("w", [F, F], f32, isOutput=False)    # W [fo, fi]
    a1_ext = nc.declare_dram_parameter("a1", [F, 1], f32, isOutput=False)
    a2_ext = nc.declare_dram_parameter("a2", [F, 1], f32, isOutput=False)
    out_ext = nc.declare_dram_parameter("out", [RPC, F], f32, isOutput=True)

    with tile.TileContext(nc) as tc, ExitStack() as ctx:
        const = ctx.enter_context(tc.tile_pool(name="const", bufs=1))
        setup = ctx.enter_context(tc.tile_pool(name="setup", bufs=3))
        ps_tp = ctx.enter_context(tc.tile_pool(name="ps_tp", bufs=3, space="PSUM"))
        ps_acc = ctx.enter_context(tc.tile_pool(name="ps_acc", bufs=2, space="PSUM"))
        adj_pool = ctx.enter_context(tc.tile_pool(name="adjp", bufs=4))
        work = ctx.enter_context(tc.tile_pool(name="work", bufs=3))
        pexp = ctx.enter_context(tc.tile_pool(name="pexp", bufs=4))
        outp = ctx.enter_context(tc.tile_pool(name="outp", bufs=2))

        ident = const.tile([128, 128], f32)
        make_identity(nc, ident)
        ident_bf = const.tile([128, 128], bf16)
        make_identity(nc, ident_bf)
        wt_sb = const.tile([F, F], f32)
        nc.sync.dma_start(out=wt_sb, in_=wt_ext[:, :])
        w_sb = const.tile([F, F], f32)
        nc.sync.dma_start(out=w_sb, in_=w_ext[:, :])
        a1_sb = const.tile([F, 1], f32)
        nc.sync.dma_start(out=a1_sb, in_=a1_ext[:, :])
        a2_sb = const.tile([F, 1], f32)
        nc.sync.dma_start(out=a2_sb, in_=a2_ext[:, :])
        ones_row = const.tile([1, 128], f32)
        nc.vector.memset(ones_row, 1.0)

        # big persistent tensors, split per column-chunk so the main loop can
        # start on chunk j as soon as its slice of setup is done
        sjbc_t = [const.tile([128, CCH], bf16, name=f"sjbc{_}") for _ in range(NCH)]
        whext_t = [const.tile([128, CCH // 128, F + 1], bf16, name=f"whext{_}") for _ in range(NCH)]
        sj_t = [const.tile([1, CCH], bf16, name=f"sj{_}") for _ in range(NCH)]
        si_cols = const.tile([128, RT], f32)             # s1 own rows, per-partition

        for jj in range(NCH):
            nc.vector.memset(whext_t[jj][:, :, F:F + 1], 1.0)

        # ---- w1 = W^T a1, w2 = W^T a2 (feature-space vectors) ----
        ps_w = ps_tp.tile([128, PSW], f32, tag="tp")
        nc.tensor.matmul(ps_w[:, 0:1], lhsT=w_sb, rhs=a1_sb, start=True, stop=True)
        nc.tensor.matmul(ps_w[:, 1:2], lhsT=w_sb, rhs=a2_sb, start=True, stop=True)
        w1c = const.tile([128, 1], f32)
        nc.vector.tensor_copy(out=w1c, in_=ps_w[:, 0:1])
        w2c = const.tile([128, 1], f32)
        nc.vector.tensor_copy(out=w2c, in_=ps_w[:, 1:2])

        # ---- own-row s1 column vectors: si = hTloc^T @ w1 ----
        for kk in range(RPC // 512):
            hTlc = setup.tile([128, 512], f32, tag="whT_c")
            nc.gpsimd.dma_start(out=hTlc,
                                in_=hTloc_ext[:, 512 * kk:512 * kk + 512])
            for m in range(4):
                t = 4 * kk + m
                ps3 = ps_acc.tile([128, F + 1], f32, tag="acc")
                nc.tensor.matmul(ps3[:, 0:1],
                                 lhsT=hTlc[:, 128 * m:128 * m + 128],
                                 rhs=w1c, start=True, stop=True)
                nc.vector.tensor_copy(out=si_cols[:, t:t + 1], in_=ps3[:, 0:1])

        # ---- stream hT chunks; build Wh tiles and s2 = hT^T @ w2 ----
        sj_dram = nc.dram_tensor("sj_stage", [NCH, CCH], bf16)
        for k in range(N // 1024):
            hTc = setup.tile([128, 1024], f32, tag="hTc")
            nc.gpsimd.dma_start(out=hTc, in_=hT_ext[:, 1024 * k:1024 * k + 1024])
            for m in range(8):
                ps2 = ps_tp.tile([128, PSW], f32, tag="tp")
                nc.tensor.matmul(ps2[:, 0:F], lhsT=hTc[:, 128 * m:128 * m + 128],
                                 rhs=wt_sb, start=True, stop=True)
                ci = 8 * k + m
                nc.vector.tensor_copy(out=whext_t[ci // 16][:, ci % 16, 0:F],
                                      in_=ps2[:, 0:F])
            for m in range(2):
                ps3 = ps_tp.tile([128, PSW], f32, tag="tp")
                nc.tensor.matmul(ps3[0:1, 0:512], lhsT=w2c,
                                 rhs=hTc[:, 512 * m:512 * m + 512],
                                 start=True, stop=True)
                off = 1024 * k + 512 * m
                nc.vector.tensor_copy(
                    out=sj_t[off // CCH][0:1, off % CCH:off % CCH + 512],
                    in_=ps3[0:1, 0:512])
            if k % 2 == 1:
                jj = k // 2
                nc.gpsimd.dma_start(out=sj_dram[jj:jj + 1, :],
                                    in_=sj_t[jj][0:1, :])
                for sl in range(4):
                    nc.gpsimd.dma_start(
                        out=sjbc_t[jj][32 * sl:32 * sl + 32, :],
                        in_=sj_dram[jj:jj + 1, :].to_broadcast([32, CCH]))

        # ---- main loop ----
        for t in range(RT):
            acc = ps_acc.tile([128, F + 1], f32, tag="acc")
            for j in range(NCH):
                adj_t = adj_pool.tile([128, CCH], f32, tag="adj")
                nc.sync.dma_start(
                    out=adj_t,
                    in_=adj_ext[128 * t:128 * t + 128, CCH * j:CCH * j + CCH])
                L = work.tile([128, CCH], f32, tag="L")
                nc.scalar.activation(out=L, in_=sjbc_t[j][:, :],
                                     func=AF.Prelu, bias=si_cols[:, t:t + 1],
                                     alpha=NEG_SLOPE)
                T = work.tile([128, CCH], bf16, tag="T")
                nc.vector.tensor_tensor(out=T, in0=L, in1=adj_t, op=AL.mult)
                for q in range(CCH // PSW):
                    tp = ps_tp.tile([128, PSW], bf16, tag="tp")
                    for s in range(PSW // 128):
                        nc.tensor.transpose(
                            tp[:, 128 * s:128 * s + 128],
                            T[:, PSW * q + 128 * s:PSW * q + 128 * s + 128],
                            ident_bf)
                    P_t = pexp.tile([128, PSW], bf16, tag="P")
                    nc.scalar.activation(out=P_t, in_=tp, func=AF.Exp)
                    for s in range(PSW // 128):
                        ci = (PSW * q + 128 * s) // 128
                        nc.tensor.matmul(
                            acc, lhsT=P_t[:, 128 * s:128 * s + 128],
                            rhs=whext_t[j][:, ci, :],
                            start=(j == 0 and q == 0 and s == 0),
                            stop=(j == NCH - 1 and q == CCH // PSW - 1
                                  and s == PSW // 128 - 1))
            rinv = outp.tile([128, 1], f32, tag="rinv")
            nc.vector.reciprocal(rinv, acc[:, F:F + 1])
            o_t = outp.tile([128, F], f32, tag="o")
            nc.vector.tensor_scalar(out=o_t, in0=acc[:, 0:F],
                                    scalar1=rinv[:, 0:1], scalar2=None,
                                    op0=AL.mult)
            nc.sync.dma_start(out=out_ext[128 * t:128 * t + 128, :], in_=o_t)

    nc.compile()
    return nc


def _get_nc():
    if "nc" not in _CACHE:
        _CACHE["nc"] = _build()
    return _CACHE["nc"]


def kernel(h, adj, W, a, _trace=False, _trace_kwargs=None):
    h = np.ascontiguousarray(np.asarray(h, dtype=np.float32))
    adj = np.ascontiguousarray(np.asarray(adj, dtype=np.float32))
    W = np.asarray(W, dtype=np.float32)
    a = np.asarray(a, dtype=np.float32)

    wt = np.ascontiguousarray(W.T)                    # [fi, fo]
    a1c = np.ascontiguousarray(a[0, :F].reshape(F, 1))
    a2c = np.ascontiguousarray(a[0, F:].reshape(F, 1))

    hT = np.ascontiguousarray(h.T)                    # [fi, n]
    nc = _get_nc()
    in_maps = []
    for c in range(NCORES):
        r0 = c * RPC
        in_maps.append({
            "adj": np.ascontiguousarray(adj[r0:r0 + RPC, :]),
            "hT": hT,
            "hT_loc": np.ascontiguousarray(hT[:, r0:r0 + RPC]),
            "wt": wt,
            "w": W,
            "a1": a1c,
            "a2": a2c,
        })
    kw = {}
    if _trace:
        kw["trace"] = True
        kw.update(_trace_kwargs or {})
    res = run_bass_kernel_spmd(nc, in_maps, core_ids=list(range(NCORES)), **kw)
    out = np.concatenate([res.results[c]["out"] for c in range(NCORES)], axis=0)
    if _trace:
        return out, res
    return out


# revision 12
# speedup vs baseline: 1.0154x; 1.0154x over previous
"""GATv2 layer kernel for Trainium2, sharded across 8 NeuronCores.

Computation (reference):
    Wh = h @ W.T                       [N, F]
    s1 = Wh @ a1, s2 = Wh @ a2         [N]
    e  = leaky_relu(s1[:,None] + s2[None,:], 0.2)
    attention = softmax(e * adj, dim=1)
    out = attention @ Wh               [N, F]

Sharding: rows (destination nodes) split across 8 cores, 1024 rows each.
Each core gets its adj row-block plus replicated h/W/a, computes its
1024x128 output block; host concatenates.

Per-core pipeline, tiled [128 rows x 2048 cols]:
    ACT : L = Prelu(SJbc + s1_row, alpha=0.2)      (per-partition bias)
    DVE : T = L * adj_tile
    PE  : transpose T 128x128 tiles -> PSUM [128,1024]
    ACT : P^T = Exp(T^T)  PSUM -> SBUF bf16        (fused evacuation)
    PE  : acc += P^T.T @ [Wh | 1] (bf16, FWL)      (ones col = softmax denom)
    DVE : out_rows = acc[:, :128] * 1/acc[:, 128]
Softmax is computed without max subtraction: scores are O(6) so exp is
safely in fp32 range, matching the reference up to fp rounding.
"""
import sys

for _p in ("/opt/trn_rl_repo", "/root/.axon_site/_ro/trn_rl_repo"):
    if _p not in sys.path:
        sys.path.insert(0, _p)

import numpy as np
from contextlib import ExitStack

from concourse import bacc, tile, mybir
from concourse.bass_utils import run_bass_kernel_spmd
from concourse.masks import make_identity

f32 = mybir.dt.float32
bf16 = mybir.dt.bfloat16
AL = mybir.AluOpType
AF = mybir.ActivationFunctionType

N = 8192
F = 128
NCORES = 8
RPC = N // NCORES          # rows per core = 1024
RT = RPC // 128            # row tiles per core = 8
CCH = 2048                 # column chunk for PRELU/MULT
NCH = N // CCH             # col chunks per row tile = 4
PSW = 1024                 # psum transpose tile width (2 banks)
NEG_SLOPE = 0.2

_CACHE = {}


def _build():
    nc = bacc.Bacc("TRN2", target_bir_lowering=False)

    adj_ext = nc.declare_dram_parameter("adj", [RPC, N], f32, isOutput=False)
    hT_ext = nc.declare_dram_parameter("hT", [F, N], f32, isOutput=False)
    hTloc_ext = nc.declare_dram_parameter("hT_loc", [F, RPC], f32, isOutput=False)
    wt_ext = nc.declare_dram_parameter("wt", [F, F], f32, isOutput=False)  # W^T [fi, fo]
    w_ext = nc.declare_dram_parameter("w", [F, F], f32, isOutput=False)    # W [fo, fi]
    a1_ext = nc.declare_dram_parameter("a1", [F, 1], f32, isOutput=False)
    a2_ext = nc.declare_dram_parameter("a2", [F, 1], f32, isOutput=False)
    out_ext = nc.declare_dram_parameter("out", [RPC, F], f32, isOutput=True)

    with tile.TileContext(nc) as tc, ExitStack() as ctx:
        const = ctx.enter_context(tc.tile_pool(name="const", bufs=1))
        setup = ctx.enter_context(tc.tile_pool(name="setup", bufs=3))
        ps_tp = ctx.enter_context(tc.tile_pool(name="ps_tp", bufs=3, space="PSUM"))
        ps_acc = ctx.enter_context(tc.tile_pool(name="ps_acc", bufs=2, space="PSUM"))
        adj_pool = ctx.enter_context(tc.tile_pool(name="adjp", bufs=4))
        work = ctx.enter_context(tc.tile_pool(name="work", bufs=3))
        pexp = ctx.enter_context(tc.tile_pool(name="pexp", bufs=4))
        outp = ctx.enter_context(tc.tile_pool(name="outp", bufs=2))

        ident = const.tile([128, 128], f32)
        make_identity(nc, ident)
        ident_bf = const.tile([128, 128], bf16)
        make_identity(nc, ident_bf)
        wt_sb = const.tile([F, F], f32)
        nc.sync.dma_start(out=wt_sb, in_=wt_ext[:, :])
        w_sb = const.tile([F, F], f32)
        nc.sync.dma_start(out=w_sb, in_=w_ext[:, :])
        a1_sb = const.tile([F, 1], f32)
        nc.sync.dma_start(out=a1_sb, in_=a1_ext[:, :])
        a2_sb = const.tile([F, 1], f32)
        nc.sync.dma_start(out=a2_sb, in_=a2_ext[:, :])
        ones_row = const.tile([1, 128], f32)
        nc.vector.memset(ones_row, 1.0)

        # big persistent tensors, split per column-chunk so the main loop can
        # start on chunk j as soon as its slice of setup is done
        sjbc_t = [const.tile([128, CCH], bf16, name=f"sjbc{_}") for _ in range(NCH)]
        whext_t = [const.tile([128, CCH // 128, F + 1], bf16, name=f"whext{_}") for _ in range(NCH)]
        sj_t = [const.tile([1, CCH], bf16, name=f"sj{_}") for _ in range(NCH)]
        si_cols = const.tile([128, RT], f32)             # s1 own rows, per-partition

        for jj in range(NCH):
            nc.vector.memset(whext_t[jj][:, :, F:F + 1], 1.0)

        # ---- w1 = W^T a1, w2 = W^T a2 (feature-space vectors) ----
        ps_w = ps_tp.tile([128, PSW], f32, tag="tp")
        nc.tensor.matmul(ps_w[:, 0:1], lhsT=w_sb, rhs=a1_sb, start=True, stop=True)
        nc.tensor.matmul(ps_w[:, 1:2], lhsT=w_sb, rhs=a2_sb, start=True, stop=True)
        w1c = const.tile([128, 1], f32)
        nc.vector.tensor_copy(out=w1c, in_=ps_w[:, 0:1])
        w2c = const.tile([128, 1], f32)
        nc.vector.tensor_copy(out=w2c, in_=ps_w[:, 1:2])

        # ---- own-row s1 column vectors: si = hTloc^T @ w1 ----
        for kk in range(RPC // 512):
            hTlc = setup.tile([128, 512], f32, tag="whT_c")
            nc.gpsimd.dma_start(out=hTlc,
                                in_=hTloc_ext[:, 512 * kk:512 * kk + 512])
            for m in range(4):
                t = 4 * kk + m
                ps3 = ps_acc.tile([128, F + 1], f32, tag="acc")
                nc.tensor.matmul(ps3[:, 0:1],
                                 lhsT=hTlc[:, 128 * m:128 * m + 128],
                                 rhs=w1c, start=True, stop=True)
                nc.vector.tensor_copy(out=si_cols[:, t:t + 1], in_=ps3[:, 0:1])

        # ---- stream hT chunks; build Wh tiles and s2 = hT^T @ w2 ----
        sj_dram = nc.dram_tensor("sj_stage", [NCH, CCH], bf16)
        for k in range(N // 1024):
            hTc = setup.tile([128, 1024], f32, tag="hTc")
            nc.gpsimd.dma_start(out=hTc, in_=hT_ext[:, 1024 * k:1024 * k + 1024])
            for m in range(8):
                ps2 = ps_tp.tile([128, PSW], f32, tag="tp")
                nc.tensor.matmul(ps2[:, 0:F], lhsT=hTc[:, 128 * m:128 * m + 128],
                                 rhs=wt_sb, start=True, stop=True)
                ci = 8 * k + m
                nc.vector.tensor_copy(out=whext_t[ci // 16][:, ci % 16, 0:F],
                                      in_=ps2[:, 0:F])
            for m in range(2):
                ps3 = ps_tp.tile([128, PSW], f32, tag="tp")
                nc.tensor.matmul(ps3[0:1, 0:512], lhsT=w2c,
                                 rhs=hTc[:, 512 * m:512 * m + 512],
                                 start=True, stop=True)
                off = 1024 * k + 512 * m
                nc.vector.tensor_copy(
                    out=sj_t[off // CCH][0:1, off % CCH:off % CCH + 512],
                    in_=ps3[0:1, 0:512])
            if k % 2 == 1:
                jj = k // 2
                nc.gpsimd.dma_start(out=sj_dram[jj:jj + 1, :],
                                    in_=sj_t[jj][0:1, :])
                nc.gpsimd.dma_start(
                    out=sjbc_t[jj],
                    in_=sj_dram[jj:jj + 1, :].to_broadcast([128, CCH]))

        # ---- main loop ----
        for t in range(RT):
            acc = ps_acc.tile([128, F + 1], f32, tag="acc")
            for j in range(NCH):
                adj_t = adj_pool.tile([128, CCH], f32, tag="adj")
                nc.sync.dma_start(
                    out=adj_t,
                    in_=adj_ext[128 * t:128 * t + 128, CCH * j:CCH * j + CCH])
                L = work.tile([128, CCH], f32, tag="L")
                nc.scalar.activation(out=L, in_=sjbc_t[j][:, :],
                                     func=AF.Prelu, bias=si_cols[:, t:t + 1],
                                     alpha=NEG_SLOPE)
                T = work.tile([128, CCH], bf16, tag="T")
                nc.vector.tensor_tensor(out=T, in0=L, in1=adj_t, op=AL.mult)
                for q in range(CCH // PSW):
                    tp = ps_tp.tile([128, PSW], bf16, tag="tp")
                    for s in range(PSW // 128):
                        nc.tensor.transpose(
                            tp[:, 128 * s:128 * s + 128],
                            T[:, PSW * q + 128 * s:PSW * q + 128 * s + 128],
                            ident_bf)
                    P_t = pexp.tile([128, PSW], bf16, tag="P")
                    nc.scalar.activation(out=P_t, in_=tp, func=AF.Exp)
                    for s in range(PSW // 128):
                        ci = (PSW * q + 128 * s) // 128
                        nc.tensor.matmul(
                            acc, lhsT=P_t[:, 128 * s:128 * s + 128],
                            rhs=whext_t[j][:, ci, :],
                            start=(j == 0 and q == 0 and s == 0),
                            stop=(j == NCH - 1 and q == CCH // PSW - 1
                                  and s == PSW // 128 - 1))
            rinv = outp.tile([128, 1], f32, tag="rinv")
            nc.vector.reciprocal(rinv, acc[:, F:F + 1])
            o_t = outp.tile([128, F], f32, tag="o")
            nc.vector.tensor_scalar(out=o_t, in0=acc[:, 0:F],
                                    scalar1=rinv[:, 0:1], scalar2=None,
                                    op0=AL.mult)
            nc.sync.dma_start(out=out_ext[128 * t:128 * t + 128, :], in_=o_t)

    nc.compile()
    return nc


def _get_nc():
    if "nc" not in _CACHE:
        _CACHE["nc"] = _build()
    return _CACHE["nc"]


def kernel(h, adj, W, a, _trace=False, _trace_kwargs=None):
    h = np.ascontiguousarray(np.asarray(h, dtype=np.float32))
    adj = np.ascontiguousarray(np.asarray(adj, dtype=np.float32))
    W = np.asarray(W, dtype=np.float32)
    a = np.asarray(a, dtype=np.float32)

    wt = np.ascontiguousarray(W.T)                    # [fi, fo]
    a1c = np.ascontiguousarray(a[0, :F].reshape(F, 1))
    a2c = np.ascontiguousarray(a[0, F:].reshape(F, 1))

    hT = np.ascontiguousarray(h.T)                    # [fi, n]
    nc = _get_nc()
    in_maps = []
    for c in range(NCORES):
        r0 = c * RPC
        in_maps.append({
            "adj": np.ascontiguousarray(adj[r0:r0 + RPC, :]),
            "hT": hT,
            "hT_loc": np.ascontiguousarray(hT[:, r0:r0 + RPC]),
            "wt": wt,
            "w": W,
            "a1": a1c,
            "a2": a2c,
        })
    kw = {}
    if _trace:
        kw["trace"] = True
        kw.update(_trace_kwargs or {})
    res = run_bass_kernel_spmd(nc, in_maps, core_ids=list(range(NCORES)), **kw)
    out = np.concatenate([res.results[c]["out"] for c in range(NCORES)], axis=0)
    if _trace:
        return out, res
    return out


# revision 13
# speedup vs baseline: 1.1835x; 1.1655x over previous
"""GATv2 layer kernel for Trainium2, sharded across 8 NeuronCores.

Computation (reference):
    Wh = h @ W.T                       [N, F]
    s1 = Wh @ a1, s2 = Wh @ a2         [N]
    e  = leaky_relu(s1[:,None] + s2[None,:], 0.2)
    attention = softmax(e * adj, dim=1)
    out = attention @ Wh               [N, F]

Sharding: rows (destination nodes) split across 8 cores, 1024 rows each.
Each core gets its adj row-block plus replicated h/W/a, computes its
1024x128 output block; host concatenates.

Per-core pipeline, tiled [128 rows x 2048 cols]:
    ACT : L = Prelu(SJbc + s1_row, alpha=0.2)      (per-partition bias)
    DVE : T = L * adj_tile
    PE  : transpose T 128x128 tiles -> PSUM [128,1024]
    ACT : P^T = Exp(T^T)  PSUM -> SBUF bf16        (fused evacuation)
    PE  : acc += P^T.T @ [Wh | 1] (bf16, FWL)      (ones col = softmax denom)
    DVE : out_rows = acc[:, :128] * 1/acc[:, 128]
Softmax is computed without max subtraction: scores are O(6) so exp is
safely in fp32 range, matching the reference up to fp rounding.
"""
import sys

for _p in ("/opt/trn_rl_repo", "/root/.axon_site/_ro/trn_rl_repo"):
    if _p not in sys.path:
        sys.path.insert(0, _p)

import numpy as np
from contextlib import ExitStack

from concourse import bacc, tile, mybir
from concourse.bass_utils import run_bass_kernel_spmd
from concourse.masks import make_identity

f32 = mybir.dt.float32
bf16 = mybir.dt.bfloat16
AL = mybir.AluOpType
AF = mybir.ActivationFunctionType

N = 8192
F = 128
NCORES = 8
RPC = N // NCORES          # rows per core = 1024
RT = RPC // 128            # row tiles per core = 8
CCH = 2048                 # column chunk for PRELU/MULT
NCH = N // CCH             # col chunks per row tile = 4
PSW = 1024                 # psum transpose tile width (2 banks)
NEG_SLOPE = 0.2

_CACHE = {}


def _build():
    nc = bacc.Bacc("TRN2", target_bir_lowering=False)

    adj_ext = nc.declare_dram_parameter("adj", [RPC, N], f32, isOutput=False)
    hT_ext = nc.declare_dram_parameter("hT", [F, N], f32, isOutput=False)
    hTloc_ext = nc.declare_dram_parameter("hT_loc", [F, RPC], f32, isOutput=False)
    wt_ext = nc.declare_dram_parameter("wt", [F, F], f32, isOutput=False)  # W^T [fi, fo]
    w_ext = nc.declare_dram_parameter("w", [F, F], f32, isOutput=False)    # W [fo, fi]
    a1_ext = nc.declare_dram_parameter("a1", [F, 1], f32, isOutput=False)
    a2_ext = nc.declare_dram_parameter("a2", [F, 1], f32, isOutput=False)
    out_ext = nc.declare_dram_parameter("out", [RPC, F], f32, isOutput=True)

    with tile.TileContext(nc) as tc, ExitStack() as ctx:
        const = ctx.enter_context(tc.tile_pool(name="const", bufs=1))
        setup = ctx.enter_context(tc.tile_pool(name="setup", bufs=3))
        ps_tp = ctx.enter_context(tc.tile_pool(name="ps_tp", bufs=3, space="PSUM"))
        ps_acc = ctx.enter_context(tc.tile_pool(name="ps_acc", bufs=2, space="PSUM"))
        adj_pool = ctx.enter_context(tc.tile_pool(name="adjp", bufs=4))
        work = ctx.enter_context(tc.tile_pool(name="work", bufs=3))
        pexp = ctx.enter_context(tc.tile_pool(name="pexp", bufs=4))
        outp = ctx.enter_context(tc.tile_pool(name="outp", bufs=2))

        ident = const.tile([128, 128], f32)
        make_identity(nc, ident)
        ident_bf = const.tile([128, 128], bf16)
        make_identity(nc, ident_bf)
        wt_sb = const.tile([F, F], f32)
        nc.sync.dma_start(out=wt_sb, in_=wt_ext[:, :])
        w_sb = const.tile([F, F], f32)
        nc.sync.dma_start(out=w_sb, in_=w_ext[:, :])
        a1_sb = const.tile([F, 1], f32)
        nc.sync.dma_start(out=a1_sb, in_=a1_ext[:, :])
        a2_sb = const.tile([F, 1], f32)
        nc.sync.dma_start(out=a2_sb, in_=a2_ext[:, :])
        ones_row = const.tile([1, 128], f32)
        nc.vector.memset(ones_row, 1.0)

        # big persistent tensors, split per column-chunk so the main loop can
        # start on chunk j as soon as its slice of setup is done
        sjbc_t = [const.tile([128, CCH], f32, name=f"sjbc{_}") for _ in range(NCH)]
        whext_t = [const.tile([128, CCH // 128, F + 1], bf16, name=f"whext{_}") for _ in range(NCH)]
        sj_t = [const.tile([1, CCH], f32, name=f"sj{_}") for _ in range(NCH)]
        si_cols = const.tile([128, RT], f32)             # s1 own rows, per-partition

        for jj in range(NCH):
            nc.vector.memset(whext_t[jj][:, :, F:F + 1], 1.0)

        # ---- w1 = W^T a1, w2 = W^T a2 (feature-space vectors) ----
        ps_w = ps_tp.tile([128, PSW], f32, tag="tp")
        nc.tensor.matmul(ps_w[:, 0:1], lhsT=w_sb, rhs=a1_sb, start=True, stop=True)
        nc.tensor.matmul(ps_w[:, 1:2], lhsT=w_sb, rhs=a2_sb, start=True, stop=True)
        w1c = const.tile([128, 1], f32)
        nc.vector.tensor_copy(out=w1c, in_=ps_w[:, 0:1])
        w2c = const.tile([128, 1], f32)
        nc.vector.tensor_copy(out=w2c, in_=ps_w[:, 1:2])

        # ---- own-row s1 column vectors: si = hTloc^T @ w1 ----
        for kk in range(RPC // 512):
            hTlc = setup.tile([128, 512], f32, tag="whT_c")
            nc.gpsimd.dma_start(out=hTlc,
                                in_=hTloc_ext[:, 512 * kk:512 * kk + 512])
            for m in range(4):
                t = 4 * kk + m
                ps3 = ps_acc.tile([128, F + 1], f32, tag="acc")
                nc.tensor.matmul(ps3[:, 0:1],
                                 lhsT=hTlc[:, 128 * m:128 * m + 128],
                                 rhs=w1c, start=True, stop=True)
                nc.vector.tensor_copy(out=si_cols[:, t:t + 1], in_=ps3[:, 0:1])

        # ---- stream hT chunks; build Wh tiles and s2 = hT^T @ w2 ----
        sj_dram = nc.dram_tensor("sj_stage", [NCH, CCH], f32)
        for k in range(N // 1024):
            hTc = setup.tile([128, 1024], f32, tag="hTc")
            nc.gpsimd.dma_start(out=hTc, in_=hT_ext[:, 1024 * k:1024 * k + 1024])
            for m in range(8):
                ps2 = ps_tp.tile([128, PSW], f32, tag="tp")
                nc.tensor.matmul(ps2[:, 0:F], lhsT=hTc[:, 128 * m:128 * m + 128],
                                 rhs=wt_sb, start=True, stop=True)
                ci = 8 * k + m
                nc.vector.tensor_copy(out=whext_t[ci // 16][:, ci % 16, 0:F],
                                      in_=ps2[:, 0:F])
            for m in range(2):
                ps3 = ps_tp.tile([128, PSW], f32, tag="tp")
                nc.tensor.matmul(ps3[0:1, 0:512], lhsT=w2c,
                                 rhs=hTc[:, 512 * m:512 * m + 512],
                                 start=True, stop=True)
                off = 1024 * k + 512 * m
                nc.vector.tensor_copy(
                    out=sj_t[off // CCH][0:1, off % CCH:off % CCH + 512],
                    in_=ps3[0:1, 0:512])
            if k % 2 == 1:
                jj = k // 2
                nc.gpsimd.dma_start(out=sj_dram[jj:jj + 1, :],
                                    in_=sj_t[jj][0:1, :])
                nc.gpsimd.dma_start(
                    out=sjbc_t[jj],
                    in_=sj_dram[jj:jj + 1, :].to_broadcast([128, CCH]))

        # ---- main loop ----
        for t in range(RT):
            acc = ps_acc.tile([128, F + 1], f32, tag="acc")
            for j in range(NCH):
                adj_t = adj_pool.tile([128, CCH], f32, tag="adj")
                nc.sync.dma_start(
                    out=adj_t,
                    in_=adj_ext[128 * t:128 * t + 128, CCH * j:CCH * j + CCH])
                L = work.tile([128, CCH], f32, tag="L")
                nc.scalar.activation(out=L, in_=sjbc_t[j][:, :],
                                     func=AF.Prelu, bias=si_cols[:, t:t + 1],
                                     alpha=NEG_SLOPE)
                T = work.tile([128, CCH], bf16, tag="T")
                nc.vector.tensor_tensor(out=T, in0=L, in1=adj_t, op=AL.mult)
                for q in range(CCH // PSW):
                    tp = ps_tp.tile([128, PSW], bf16, tag="tp")
                    for s in range(PSW // 128):
                        nc.tensor.transpose(
                            tp[:, 128 * s:128 * s + 128],
                            T[:, PSW * q + 128 * s:PSW * q + 128 * s + 128],
                            ident_bf)
                    P_t = pexp.tile([128, PSW], bf16, tag="P")
                    nc.scalar.activation(out=P_t, in_=tp, func=AF.Exp)
                    for s in range(PSW // 128):
                        ci = (PSW * q + 128 * s) // 128
                        nc.tensor.matmul(
                            acc, lhsT=P_t[:, 128 * s:128 * s + 128],
                            rhs=whext_t[j][:, ci, :],
                            start=(j == 0 and q == 0 and s == 0),
                            stop=(j == NCH - 1 and q == CCH // PSW - 1
                                  and s == PSW // 128 - 1))
            rinv = outp.tile([128, 1], f32, tag="rinv")
            nc.vector.reciprocal(rinv, acc[:, F:F + 1])
            o_t = outp.tile([128, F], f32, tag="o")
            nc.vector.tensor_scalar(out=o_t, in0=acc[:, 0:F],
                                    scalar1=rinv[:, 0:1], scalar2=None,
                                    op0=AL.mult)
            nc.sync.dma_start(out=out_ext[128 * t:128 * t + 128, :], in_=o_t)

    nc.compile()
    return nc


def _get_nc():
    if "nc" not in _CACHE:
        _CACHE["nc"] = _build()
    return _CACHE["nc"]


def kernel(h, adj, W, a, _trace=False, _trace_kwargs=None):
    h = np.ascontiguousarray(np.asarray(h, dtype=np.float32))
    adj = np.ascontiguousarray(np.asarray(adj, dtype=np.float32))
    W = np.asarray(W, dtype=np.float32)
    a = np.asarray(a, dtype=np.float32)

    wt = np.ascontiguousarray(W.T)                    # [fi, fo]
    a1c = np.ascontiguousarray(a[0, :F].reshape(F, 1))
    a2c = np.ascontiguousarray(a[0, F:].reshape(F, 1))

    hT = np.ascontiguousarray(h.T)                    # [fi, n]
    nc = _get_nc()
    in_maps = []
    for c in range(NCORES):
        r0 = c * RPC
        in_maps.append({
            "adj": np.ascontiguousarray(adj[r0:r0 + RPC, :]),
            "hT": hT,
            "hT_loc": np.ascontiguousarray(hT[:, r0:r0 + RPC]),
            "wt": wt,
            "w": W,
            "a1": a1c,
            "a2": a2c,
        })
    kw = {}
    if _trace:
        kw["trace"] = True
        kw.update(_trace_kwargs or {})
    res = run_bass_kernel_spmd(nc, in_maps, core_ids=list(range(NCORES)), **kw)
    out = np.concatenate([res.results[c]["out"] for c in range(NCORES)], axis=0)
    if _trace:
        return out, res
    return out


# revision 14
# speedup vs baseline: 1.2591x; 1.0639x over previous
"""GATv2 layer kernel for Trainium2, sharded across 8 NeuronCores.

Computation (reference):
    Wh = h @ W.T                       [N, F]
    s1 = Wh @ a1, s2 = Wh @ a2         [N]
    e  = leaky_relu(s1[:,None] + s2[None,:], 0.2)
    attention = softmax(e * adj, dim=1)
    out = attention @ Wh               [N, F]

Sharding: rows (destination nodes) split across 8 cores, 1024 rows each.
Each core gets its adj row-block plus replicated h/W/a, computes its
1024x128 output block; host concatenates.

Per-core pipeline, tiled [128 rows x 2048 cols]:
    ACT : L = Prelu(SJbc + s1_row, alpha=0.2)      (per-partition bias)
    DVE : T = L * adj_tile
    PE  : transpose T 128x128 tiles -> PSUM [128,1024]
    ACT : P^T = Exp(T^T)  PSUM -> SBUF bf16        (fused evacuation)
    PE  : acc += P^T.T @ [Wh | 1] (bf16, FWL)      (ones col = softmax denom)
    DVE : out_rows = acc[:, :128] * 1/acc[:, 128]
Softmax is computed without max subtraction: scores are O(6) so exp is
safely in fp32 range, matching the reference up to fp rounding.
"""
import sys

for _p in ("/opt/trn_rl_repo", "/root/.axon_site/_ro/trn_rl_repo"):
    if _p not in sys.path:
        sys.path.insert(0, _p)

import numpy as np
from contextlib import ExitStack

from concourse import bacc, tile, mybir
from concourse.bass_utils import run_bass_kernel_spmd
from concourse.masks import make_identity

f32 = mybir.dt.float32
bf16 = mybir.dt.bfloat16
AL = mybir.AluOpType
AF = mybir.ActivationFunctionType

N = 8192
F = 128
NCORES = 8
RPC = N // NCORES          # rows per core = 1024
RT = RPC // 128            # row tiles per core = 8
CCH = 2048                 # column chunk for PRELU/MULT
NCH = N // CCH             # col chunks per row tile = 4
PSW = 1024                 # psum transpose tile width (2 banks)
NEG_SLOPE = 0.2

_CACHE = {}


def _build():
    nc = bacc.Bacc("TRN2", target_bir_lowering=False)

    adj_ext = nc.declare_dram_parameter("adj", [RPC, N], f32, isOutput=False)
    hT_ext = nc.declare_dram_parameter("hT", [F, N], f32, isOutput=False)
    hTloc_ext = nc.declare_dram_parameter("hT_loc", [F, RPC], f32, isOutput=False)
    wt_ext = nc.declare_dram_parameter("wt", [F, F], f32, isOutput=False)  # W^T [fi, fo]
    w_ext = nc.declare_dram_parameter("w", [F, F], f32, isOutput=False)    # W [fo, fi]
    a1_ext = nc.declare_dram_parameter("a1", [F, 1], f32, isOutput=False)
    a2_ext = nc.declare_dram_parameter("a2", [F, 1], f32, isOutput=False)
    out_ext = nc.declare_dram_parameter("out", [RPC, F], f32, isOutput=True)

    with tile.TileContext(nc) as tc, ExitStack() as ctx:
        const = ctx.enter_context(tc.tile_pool(name="const", bufs=1))
        setup = ctx.enter_context(tc.tile_pool(name="setup", bufs=3))
        ps_tp = ctx.enter_context(tc.tile_pool(name="ps_tp", bufs=3, space="PSUM"))
        ps_acc = ctx.enter_context(tc.tile_pool(name="ps_acc", bufs=2, space="PSUM"))
        adj_pool = ctx.enter_context(tc.tile_pool(name="adjp", bufs=4))
        work = ctx.enter_context(tc.tile_pool(name="work", bufs=3))
        pexp = ctx.enter_context(tc.tile_pool(name="pexp", bufs=4))
        outp = ctx.enter_context(tc.tile_pool(name="outp", bufs=2))

        ident = const.tile([128, 128], f32)
        make_identity(nc, ident)
        ident_bf = const.tile([128, 128], bf16)
        make_identity(nc, ident_bf)
        wt_sb = const.tile([F, F], f32)
        nc.sync.dma_start(out=wt_sb, in_=wt_ext[:, :])
        w_sb = const.tile([F, F], f32)
        nc.sync.dma_start(out=w_sb, in_=w_ext[:, :])
        a1_sb = const.tile([F, 1], f32)
        nc.sync.dma_start(out=a1_sb, in_=a1_ext[:, :])
        a2_sb = const.tile([F, 1], f32)
        nc.sync.dma_start(out=a2_sb, in_=a2_ext[:, :])
        ones_row = const.tile([1, 128], f32)
        nc.vector.memset(ones_row, 1.0)

        # big persistent tensors, split per column-chunk so the main loop can
        # start on chunk j as soon as its slice of setup is done
        sjbc_t = [const.tile([128, CCH], f32, name=f"sjbc{_}") for _ in range(NCH)]
        whext_t = [const.tile([128, CCH // 128, F + 1], bf16, name=f"whext{_}") for _ in range(NCH)]
        sj_t = [const.tile([1, CCH], f32, name=f"sj{_}") for _ in range(NCH)]
        si_cols = const.tile([128, RT], f32)             # s1 own rows, per-partition

        for jj in range(NCH):
            nc.vector.memset(whext_t[jj][:, :, F:F + 1], 1.0)

        # ---- w1 = W^T a1, w2 = W^T a2 (feature-space vectors) ----
        ps_w = ps_tp.tile([128, PSW], f32, tag="tp")
        nc.tensor.matmul(ps_w[:, 0:1], lhsT=w_sb, rhs=a1_sb, start=True, stop=True)
        nc.tensor.matmul(ps_w[:, 1:2], lhsT=w_sb, rhs=a2_sb, start=True, stop=True)
        w1c = const.tile([128, 1], f32)
        nc.vector.tensor_copy(out=w1c, in_=ps_w[:, 0:1])
        w2c = const.tile([128, 1], f32)
        nc.vector.tensor_copy(out=w2c, in_=ps_w[:, 1:2])

        # ---- own-row s1 column vectors: si = hTloc^T @ w1 ----
        for kk in range(RPC // 512):
            hTlc = setup.tile([128, 512], f32, tag="whT_c")
            nc.sync.dma_start(out=hTlc,
                              in_=hTloc_ext[:, 512 * kk:512 * kk + 512])
            for m in range(4):
                t = 4 * kk + m
                ps3 = ps_acc.tile([128, F + 1], f32, tag="acc")
                nc.tensor.matmul(ps3[:, 0:1],
                                 lhsT=hTlc[:, 128 * m:128 * m + 128],
                                 rhs=w1c, start=True, stop=True)
                nc.vector.tensor_copy(out=si_cols[:, t:t + 1], in_=ps3[:, 0:1])

        # ---- stream hT chunks; build Wh tiles and s2 = hT^T @ w2 ----
        sj_dram = nc.dram_tensor("sj_stage", [NCH, CCH], f32)
        for k in range(N // 1024):
            hTc = setup.tile([128, 1024], f32, tag="hTc")
            nc.sync.dma_start(out=hTc, in_=hT_ext[:, 1024 * k:1024 * k + 1024])
            for m in range(8):
                ps2 = ps_tp.tile([128, PSW], f32, tag="tp")
                nc.tensor.matmul(ps2[:, 0:F], lhsT=hTc[:, 128 * m:128 * m + 128],
                                 rhs=wt_sb, start=True, stop=True)
                ci = 8 * k + m
                nc.vector.tensor_copy(out=whext_t[ci // 16][:, ci % 16, 0:F],
                                      in_=ps2[:, 0:F])
            for m in range(2):
                ps3 = ps_tp.tile([128, PSW], f32, tag="tp")
                nc.tensor.matmul(ps3[0:1, 0:512], lhsT=w2c,
                                 rhs=hTc[:, 512 * m:512 * m + 512],
                                 start=True, stop=True)
                off = 1024 * k + 512 * m
                nc.vector.tensor_copy(
                    out=sj_t[off // CCH][0:1, off % CCH:off % CCH + 512],
                    in_=ps3[0:1, 0:512])
            if k % 2 == 1:
                jj = k // 2
                nc.gpsimd.dma_start(out=sj_dram[jj:jj + 1, :],
                                    in_=sj_t[jj][0:1, :])
                nc.gpsimd.dma_start(
                    out=sjbc_t[jj],
                    in_=sj_dram[jj:jj + 1, :].to_broadcast([128, CCH]))

        # ---- main loop ----
        for t in range(RT):
            acc = ps_acc.tile([128, F + 1], f32, tag="acc")
            for j in range(NCH):
                adj_t = adj_pool.tile([128, CCH], f32, tag="adj")
                nc.sync.dma_start(
                    out=adj_t,
                    in_=adj_ext[128 * t:128 * t + 128, CCH * j:CCH * j + CCH])
                ch = t * NCH + j
                if (((ch + 1) * 14) // 32) > ((ch * 14) // 32):
                    # DVE path: fused (sjbc+si)*adj then leaky via stt
                    T0 = work.tile([128, CCH], f32, tag="L")
                    nc.vector.scalar_tensor_tensor(
                        out=T0, in0=sjbc_t[j][:, :],
                        scalar=si_cols[:, t:t + 1], in1=adj_t,
                        op0=AL.add, op1=AL.mult)
                    T = work.tile([128, CCH], bf16, tag="T")
                    nc.vector.scalar_tensor_tensor(
                        out=T, in0=T0, scalar=NEG_SLOPE, in1=T0,
                        op0=AL.mult, op1=AL.max)
                else:
                    L = work.tile([128, CCH], f32, tag="L")
                    nc.scalar.activation(out=L, in_=sjbc_t[j][:, :],
                                         func=AF.Prelu, bias=si_cols[:, t:t + 1],
                                         alpha=NEG_SLOPE)
                    T = work.tile([128, CCH], bf16, tag="T")
                    nc.vector.tensor_tensor(out=T, in0=L, in1=adj_t, op=AL.mult)
                for q in range(CCH // PSW):
                    tp = ps_tp.tile([128, PSW], bf16, tag="tp")
                    for s in range(PSW // 128):
                        nc.tensor.transpose(
                            tp[:, 128 * s:128 * s + 128],
                            T[:, PSW * q + 128 * s:PSW * q + 128 * s + 128],
                            ident_bf)
                    P_t = pexp.tile([128, PSW], bf16, tag="P")
                    nc.scalar.activation(out=P_t, in_=tp, func=AF.Exp)
                    for s in range(PSW // 128):
                        ci = (PSW * q + 128 * s) // 128
                        nc.tensor.matmul(
                            acc, lhsT=P_t[:, 128 * s:128 * s + 128],
                            rhs=whext_t[j][:, ci, :],
                            start=(j == 0 and q == 0 and s == 0),
                            stop=(j == NCH - 1 and q == CCH // PSW - 1
                                  and s == PSW // 128 - 1))
            rinv = outp.tile([128, 1], f32, tag="rinv")
            nc.vector.reciprocal(rinv, acc[:, F:F + 1])
            o_t = outp.tile([128, F], f32, tag="o")
            nc.vector.tensor_scalar(out=o_t, in0=acc[:, 0:F],
                                    scalar1=rinv[:, 0:1], scalar2=None,
                                    op0=AL.mult)
            nc.sync.dma_start(out=out_ext[128 * t:128 * t + 128, :], in_=o_t)

    nc.compile()
    return nc


def _get_nc():
    if "nc" not in _CACHE:
        _CACHE["nc"] = _build()
    return _CACHE["nc"]


def kernel(h, adj, W, a, _trace=False, _trace_kwargs=None):
    h = np.ascontiguousarray(np.asarray(h, dtype=np.float32))
    adj = np.ascontiguousarray(np.asarray(adj, dtype=np.float32))
    W = np.asarray(W, dtype=np.float32)
    a = np.asarray(a, dtype=np.float32)

    wt = np.ascontiguousarray(W.T)                    # [fi, fo]
    a1c = np.ascontiguousarray(a[0, :F].reshape(F, 1))
    a2c = np.ascontiguousarray(a[0, F:].reshape(F, 1))

    hT = np.ascontiguousarray(h.T)                    # [fi, n]
    nc = _get_nc()
    in_maps = []
    for c in range(NCORES):
        r0 = c * RPC
        in_maps.append({
            "adj": np.ascontiguousarray(adj[r0:r0 + RPC, :]),
            "hT": hT,
            "hT_loc": np.ascontiguousarray(hT[:, r0:r0 + RPC]),
            "wt": wt,
            "w": W,
            "a1": a1c,
            "a2": a2c,
        })
    kw = {}
    if _trace:
        kw["trace"] = True
        kw.update(_trace_kwargs or {})
    res = run_bass_kernel_spmd(nc, in_maps, core_ids=list(range(NCORES)), **kw)
    out = np.concatenate([res.results[c]["out"] for c in range(NCORES)], axis=0)
    if _trace:
        return out, res
    return out
